# revision 23
# baseline (speedup 1.0000x reference)
"""Multi-head attention (B=4, S=2048, D=1024, H=16) on 8 TRN2 NeuronCores.

Sharding: DP=4 over batch x TP=2 over heads. Core c handles batch c//2 and
heads 8*(c%2) .. 8*(c%2)+8. Each core computes a partial output [S, D] (its
heads' contribution to the out-projection); the host sums the two TP partials
per batch and adds the output bias.

Key compaction: the key-padding mask removes ~half the keys, so the host
gathers unmasked key rows per batch (padded to a multiple of 128). k/v
projections and attention only touch NKV ~= S/2 keys; padding keys carry a
big negative additive bias fused into the exp so they contribute exactly 0.

On-chip layouts (all matmul operands bf16, accumulation fp32 in PSUM):
  qT/kT : [hd, seq] with the two heads of a pair stacked on partitions
          (0-63 / 64-127) -> the scores matmuls (K=64) auto-pack into PE
          row-groups and run concurrently.  qT is PRE-SCALED by
          ALPHA = SCALE * 128/ln2 so PSUM scores are in "bf16-bits" units.
  exp   : split across two engines so neither paces the kernel. ScalarE
          computes exact exp via its free affine (scale=1/A, bias=mask);
          for kc%3==2 VectorE computes a Schraudolph exp in ONE op:
          bits16 = max(scores + B, 0) converted to int16 and bit-viewed as
          bf16 (linear-in-log2 approx, ~1.8% rms sawtooth, softmax-scale
          invariant; masked keys land at exactly +0 -> exp 0).
  v_aug : [keys, v | ones(64)] -> the ctx matmul accumulates ctxT (rows 0-63)
          and the softmax denominator replicated across rows 64-127, so the
          normalization is a fast-reciprocal + multiply on VectorE.

Engine-queue discipline (queues are strict FIFO, so emission order is the
schedule): the exp-paced kc loop leaves ~0.5us/kc of PE slack, which is
filled by a global queue of deferred matmul chunks (remaining projections,
second v half, next q window, previous step's out-projection). force()
emits any still-queued unit a step depends on before that step's scores, so
FIFO deadlock is impossible. Input DMAs are issued from both the Sync and
GpSimd queues (each dma_start costs ~0.6us of issue time on its queue).
"""

import sys

sys.path.insert(0, "/opt/trn_rl_repo")

import numpy as np
import ml_dtypes

B, S, D, H = 4, 2048, 1024, 16
HD = D // H
SCALE = 1.0 / float(np.sqrt(HD))
NEG = -1e9

# Schraudolph exp-as-int16 constants (bf16 bit layout: 7 mantissa bits)
EXP_A = 128.0 / float(np.log(2.0))      # d bits / d ln(x)
ALPHA = EXP_A * SCALE                   # folded into wq on the host
EXP_B = 128.0 * (127.0 - 0.0573) + 0.5  # bits offset (softmax-invariant)
NEG_DVE = -1.0e6
USE_GP_DMA = True

DP = 4  # batch shards
TP = 2  # head-group shards
HL = H // TP  # heads per core (8)
DL = HL * HD  # local head dims per core (512)
N_HP = HL // 2  # head pairs per core (4)
QCH = 512  # q chunk (free dim of score matmuls)
KV_P = 128  # key chunk (partition dim of scoresT)
KC8 = D // 128  # contraction chunks for projections (8)

bf16 = ml_dtypes.bfloat16


def _windows(n, w=512):
    out = []
    off = 0
    while off < n:
        out.append((off, min(w, n - off)))
        off += w
    return out


def _build(nkv, with_bias=True):
    from concourse import bacc
    import concourse.mybir as mybir
    from concourse.tile import TileContext

    dt = mybir.dt
    f32 = dt.float32
    b16 = dt.bfloat16
    i16 = dt.int16
    EXP = mybir.ActivationFunctionType.Exp
    ADD = mybir.AluOpType.add
    MAX = mybir.AluOpType.max

    nkc = nkv // KV_P  # key chunks
    kwins = _windows(nkv)  # kt projection windows
    W1 = KC8 * 128  # weight cols per hp block

    nc = bacc.Bacc(trn_type="TRN2")

    xq_d = nc.dram_tensor("xq", (128, KC8 * S), b16, kind="ExternalInput").ap()
    xkv_d = nc.dram_tensor("xkv", (128, KC8 * nkv), b16, kind="ExternalInput").ap()
    wq_d = nc.dram_tensor("wqt", (128, N_HP * W1), b16, kind="ExternalInput").ap()
    wk_d = nc.dram_tensor("wkt", (128, N_HP * W1), b16, kind="ExternalInput").ap()
    wv_d = nc.dram_tensor("wvt", (128, KC8 * DL), b16, kind="ExternalInput").ap()
    if with_bias:
        bq_d = nc.dram_tensor("bq", (1, DL), b16, kind="ExternalInput").ap()
        bk_d = nc.dram_tensor("bk", (1, DL), b16, kind="ExternalInput").ap()
        bv_d = nc.dram_tensor("bv", (1, DL), b16, kind="ExternalInput").ap()
    wo_d = nc.dram_tensor("wot", (128, (DL // 128) * D), b16, kind="ExternalInput").ap()
    mba_d = nc.dram_tensor("mbact", (nkv,), f32, kind="ExternalInput").ap()
    mbd_d = nc.dram_tensor("mbdve", (nkv,), f32, kind="ExternalInput").ap()
    out_d = nc.dram_tensor("out", (S, D), b16, kind="ExternalOutput").ap()

    with TileContext(nc) as tc:
        with (
            tc.tile_pool(name="persist", bufs=1) as pp,
            tc.tile_pool(name="ps_s", bufs=2, space="PSUM") as ps_s,
            tc.tile_pool(name="ps_cc", bufs=1, space="PSUM") as ps_cc,
            tc.tile_pool(name="ps_aux", bufs=2, space="PSUM") as ps_aux,
            tc.tile_pool(name="etile", bufs=12) as ep,
            tc.tile_pool(name="work", bufs=2) as wp,
            tc.tile_pool(name="ob", bufs=3) as obp,
        ):
            # ---- persistent SBUF tensors ----
            xq_sb = pp.tile([128, KC8, S], b16, tag="xq")
            xq1_sb = pp.tile([1, S], b16, tag="xq1")
            xkv_sb = pp.tile([128, KC8, nkv], b16, tag="xkv")
            xkv1_sb = pp.tile([1, nkv], b16, tag="xkv1")
            wq_sb = pp.tile([128, N_HP, KC8, 128], b16, tag="wq")
            wk_sb = pp.tile([128, N_HP, KC8, 128], b16, tag="wk")
            wv_sb = pp.tile([128, KC8, DL], b16, tag="wv")
            wq1_sb = pp.tile([1, DL], b16, tag="wq1")
            wk1_sb = pp.tile([1, DL], b16, tag="wk1")
            wv1_sb = pp.tile([1, DL], b16, tag="wv1")
            wo_sb = pp.tile([128, DL // 128, D], b16, tag="wo")
            mba_sb = pp.tile([128, nkc], f32, tag="mba")
            mbd_sb = pp.tile([128, nkc], f32, tag="mbd")
            qt_sb = pp.tile([128, N_HP, S], b16, tag="qt")
            kt_sb = pp.tile([128, N_HP, nkv], b16, tag="kt")
            v_sb = pp.tile([128, nkc, HL, 128], b16, tag="v")
            ctx_sb = pp.tile([128, N_HP, S], b16, tag="ctx")

            # ---- DMA staging, split across the Sync and GpSimd issue
            # queues; ordered so the first score matmuls gate on ~2.5MB ----
            gp = nc.gpsimd if USE_GP_DMA else nc.sync
            gp.dma_start(mba_sb[:], mba_d.rearrange("(kc p) -> p kc", p=128))
            gp.dma_start(mbd_sb[:], mbd_d.rearrange("(kc p) -> p kc", p=128))

            def load_w(eng, dst, src, hp):
                eng.dma_start(
                    dst[:, hp].rearrange("p kc e -> p (kc e)"),
                    src[:, hp * W1 : hp * W1 + W1],
                )

            def load_x_win(eng, dst, src, tot, off, n):
                for kc in range(KC8):
                    eng.dma_start(
                        dst[:, kc, off : off + n],
                        src[:, kc * tot + off : kc * tot + off + n],
                    )

            nc.sync.dma_start(
                wq_sb[:, 0].rearrange("p kc e -> p (kc e)"), wq_d[:, 0:W1]
            )
            load_w(gp, wk_sb, wk_d, 0)
            load_x_win(nc.sync, xq_sb, xq_d, S, 0, 512)
            load_x_win(gp, xkv_sb, xkv_d, nkv, 0, min(512, nkv))
            if with_bias:
                gp.dma_start(wq1_sb[:], bq_d)
                gp.dma_start(wk1_sb[:], bk_d)
                gp.dma_start(wv1_sb[:], bv_d)
            for kc in range(KC8):  # v-proj weights
                nc.sync.dma_start(wv_sb[:, kc, :], wv_d[:, kc * DL : kc * DL + DL])
            for off, n in kwins[1:]:
                load_x_win(gp, xkv_sb, xkv_d, nkv, off, n)
            for kc in range(DL // 128):  # wo needed by t=1
                nc.sync.dma_start(wo_sb[:, kc, :], wo_d[:, kc * D : kc * D + D])
            for hp in range(1, N_HP):
                load_w(gp, wk_sb, wk_d, hp)
                load_w(gp, wq_sb, wq_d, hp)
            for off, n in _windows(S)[1:]:
                load_x_win(nc.sync, xq_sb, xq_d, S, off, n)

            # constants
            if with_bias:
                nc.vector.memset(xq1_sb[:], 1.0)
                nc.vector.memset(xkv1_sb[:], 1.0)
            nc.vector.memset(v_sb[:, :, :, 64:128], 1.0)

            # ================= deferred-unit machinery =================
            # unit = list of chunk thunks (each ~0.4-0.9us of PE work);
            # fill(n) emits n chunks from the queue head; force(key) emits
            # a whole unit immediately (dependency safety).
            units = {}     # key -> list of remaining thunks
            queue = []     # ordered keys
            started = set()  # units with some chunks already emitted (their
            # PSUM aux tile is live; never interleave another unit's chunks
            # before they finish)

            def add_unit(key, thunks, front=False):
                units[key] = list(thunks)
                if front:
                    pos = 1 if (queue and queue[0] in started) else 0
                    queue.insert(pos, key)
                else:
                    queue.append(key)

            def fill(n):
                done = 0
                while done < n and queue:
                    key = queue[0]
                    th = units[key]
                    if th:
                        th.pop(0)()
                        started.add(key)
                        done += 1
                    if not th:
                        queue.pop(0)
                        started.discard(key)
                        del units[key]

            def force(key):
                if key in units:
                    for th in units[key]:
                        th()
                    units[key].clear()
                    if key in queue:
                        queue.remove(key)
                    started.discard(key)
                    del units[key]

            # ---- projection unit builders ----
            def proj_qk_unit(w_sb, w1_sb, dst_sb, hp, off, n, nfree):
                """chunks computing dst[:, hp, off:off+n]"""
                x_sb, x1_sb = (xq_sb, xq1_sb) if nfree == S else (xkv_sb, xkv1_sb)
                state = {}

                def mm2(kc0):
                    def th():
                        if "ps" not in state:
                            state["ps"] = ps_aux.tile([128, 512], f32, tag="aux", name="auxps")
                        ps = state["ps"]
                        for kc in (kc0, kc0 + 1):
                            nc.tensor.matmul(
                                ps[:, :n],
                                lhsT=w_sb[:, hp, kc, :],
                                rhs=x_sb[:, kc, off : off + n],
                                start=(kc == 0),
                                stop=(not with_bias and kc == KC8 - 1),
                            )

                    return th

                def tail():
                    ps = state["ps"]
                    if with_bias:
                        nc.tensor.matmul(
                            ps[:, :n],
                            lhsT=w1_sb[:, hp * 128 : hp * 128 + 128],
                            rhs=x1_sb[:, off : off + n],
                            start=False,
                            stop=True,
                        )
                    nc.scalar.copy(out=dst_sb[:, hp, off : off + n], in_=ps[:, :n])

                return [mm2(0), mm2(2), mm2(4), lambda: (mm2(6)(), tail())]

            def v_unit(mt, half):
                """v[keys mt*128:+128, heads half*4..+4] into v_sb."""
                hs = slice(half * 256, half * 256 + 256)
                state = {}

                def mm4(kc0):
                    def th():
                        if "ps" not in state:
                            state["ps"] = ps_aux.tile([128, 512], f32, tag="aux", name="auxps")
                        ps = state["ps"]
                        for kc in range(kc0, kc0 + 4):
                            nc.tensor.matmul(
                                ps[:, 0:256],
                                lhsT=xkv_sb[:, kc, mt * 128 : mt * 128 + 128],
                                rhs=wv_sb[:, kc, hs],
                                start=(kc == 0),
                                stop=(not with_bias and kc == KC8 - 1),
                            )

                    return th

                def tail():
                    ps = state["ps"]
                    if with_bias:
                        nc.tensor.matmul(
                            ps[:, 0:256],
                            lhsT=xkv1_sb[:, mt * 128 : mt * 128 + 128],
                            rhs=wv1_sb[:, hs],
                            start=False,
                            stop=True,
                        )
                    nc.scalar.copy(
                        out=v_sb[:, mt, half * 4 : half * 4 + 4, 0:64],
                        in_=ps[:, 0:256].rearrange("p (h e) -> p h e", h=4),
                    )

                return [mm4(0), lambda: (mm4(4)(), tail())]

            def op_unit(rt):
                """out-projection for row-tile rt: 2 psum halves + copies."""
                rs = slice(rt * 128, rt * 128 + 128)
                state = {}

                def mm2(nj, k0):
                    def th():
                        key = f"ps{nj}"
                        if key not in state:
                            state[key] = ps_aux.tile([128, 512], f32, tag="aux", name="auxps")
                        ps = state[key]
                        ns = slice(nj * 512, nj * 512 + 512)
                        for khp in (k0, k0 + 1):
                            nc.tensor.matmul(
                                ps[:],
                                lhsT=ctx_sb[:, khp, rs],
                                rhs=wo_sb[:, khp, ns],
                                start=(khp == 0),
                                stop=(khp == N_HP - 1),
                            )

                    return th

                def copy(nj):
                    if "ob" not in state:
                        state["ob"] = obp.tile([128, D], b16, tag="ob", name="obt")
                    nc.scalar.copy(
                        out=state["ob"][:, nj * 512 : nj * 512 + 512],
                        in_=state[f"ps{nj}"][:],
                    )

                def tail():
                    mm2(1, 2)()
                    copy(1)
                    nc.sync.dma_start(out_d[rs, :], state["ob"][:])

                # khp 0,1 chunks first: the khp 2,3 halves wait on the
                # previous step's norm, which runs at the start of this
                # step on the otherwise-idle VectorE
                return [
                    mm2(0, 0),
                    mm2(1, 0),
                    lambda: (mm2(0, 2)(), copy(0)),
                    tail,
                ]

            # tail out-projection for the last q-chunk, split in two phases:
            # phase 1 (khp 0,1) depends only on earlier steps and keeps the
            # PE warm while the last norm chain runs; phase 2 (khp 2,3 +
            # copy + DMA) waits on the final normalizations.  PSUM comes
            # from the s-pool ([128,1024] tiles, free once exps are done).
            op_tail_state = {}

            def op_tail_p1(rt):
                rs = slice(rt * 128, rt * 128 + 128)

                def th():
                    ps = ps_s.tile([128, D], f32, tag="s", name="opt")
                    op_tail_state[rt] = ps
                    for nj in range(D // 512):
                        ns = slice(nj * 512, nj * 512 + 512)
                        for khp in (0, 1):
                            nc.tensor.matmul(
                                ps[:, ns],
                                lhsT=ctx_sb[:, khp, rs],
                                rhs=wo_sb[:, khp, ns],
                                start=(khp == 0),
                                stop=False,
                            )

                return [th]

            def op_tail_p2(rt):
                rs = slice(rt * 128, rt * 128 + 128)

                def th():
                    ps = op_tail_state[rt]
                    for nj in range(D // 512):
                        ns = slice(nj * 512, nj * 512 + 512)
                        for khp in (2, 3):
                            nc.tensor.matmul(
                                ps[:, ns],
                                lhsT=ctx_sb[:, khp, rs],
                                rhs=wo_sb[:, khp, ns],
                                start=False,
                                stop=(khp == N_HP - 1),
                            )
                    ob = obp.tile([128, D], b16, tag="ob", name="obt")
                    nc.scalar.copy(out=ob[:], in_=ps[:])
                    nc.sync.dma_start(out_d[rs, :], ob[:])

                return [th]

            def q_key(hp, qc):
                return ("q", hp, qc)

            def k_key(hp, w):
                return ("k", hp, w)

            def v_key(mt, half):
                return ("v", mt, half)

            # prologue: only what gates the first exp
            force_emit = proj_qk_unit(wq_sb, wq1_sb, qt_sb, 0, 0, 512, S)
            for th in force_emit:
                th()
            kw0 = proj_qk_unit(wk_sb, wk1_sb, kt_sb, 0, 0, kwins[0][1], nkv)
            for th in kw0:
                th()

            # queue: rest of kt hp0, v half0, then hp1.. (force() is the net)
            for wi, (off, n) in enumerate(kwins[1:], start=1):
                add_unit(k_key(0, wi), proj_qk_unit(wk_sb, wk1_sb, kt_sb, 0, off, n, nkv))
            for mt in range(nkc):
                add_unit(v_key(mt, 0), v_unit(mt, 0))
            for hp in range(1, N_HP):
                add_unit(q_key(hp, 0), proj_qk_unit(wq_sb, wq1_sb, qt_sb, hp, 0, 512, S))
                for wi, (off, n) in enumerate(kwins):
                    add_unit(
                        k_key(hp, wi),
                        proj_qk_unit(wk_sb, wk1_sb, kt_sb, hp, off, n, nkv),
                    )
                if hp == 1:
                    for mt in range(nkc):
                        add_unit(v_key(mt, 1), v_unit(mt, 1))

            # ================= attention steps =================
            for t in range(4 * N_HP):
                qc, hp = divmod(t, N_HP)
                qs = slice(qc * QCH, qc * QCH + QCH)
                half = hp // 2

                # dependency safety: everything this step reads must be
                # emitted before its consumers
                force(q_key(hp, qc))
                for wi in range(len(kwins)):
                    force(k_key(hp, wi))
                for mt in range(nkc):
                    force(v_key(mt, half))

                # out-projection for row-tile (qc-1, hp): all head-pairs of
                # q-chunk qc-1 are complete by now; its chunks go to the
                # queue front to fill this step's kc-loop PE slack
                if qc > 0:
                    rt = (qc - 1) * N_HP + hp
                    add_unit(("op", rt), op_unit(rt), front=True)

                cc = ps_cc.tile([128, 2 * QCH], f32, tag="cc")
                c0 = cc[:, 0:QCH]
                c1 = cc[:, QCH : 2 * QCH]

                def ctx_mm(ekc, hp=hp, c0=c0, c1=c1):
                    e01_p, kc_p = ekc
                    nc.tensor.matmul(
                        c0,
                        lhsT=v_sb[:, kc_p, 2 * hp, :],
                        rhs=e01_p[:, 0:QCH],
                        start=(kc_p == 0),
                        stop=(kc_p == nkc - 1),
                    )
                    nc.tensor.matmul(
                        c1,
                        lhsT=v_sb[:, kc_p, 2 * hp + 1, :],
                        rhs=e01_p[:, QCH : 2 * QCH],
                        start=(kc_p == 0),
                        stop=(kc_p == nkc - 1),
                    )

                depth = nkc if t == 0 else 4
                pending = []
                for kc in range(nkc):
                    ks = slice(kc * KV_P, kc * KV_P + KV_P)
                    fill(1)
                    s01 = ps_s.tile([128, 2 * QCH], f32, tag="s")
                    nc.tensor.matmul(
                        s01[:, 0:QCH],
                        lhsT=kt_sb[0:64, hp, ks],
                        rhs=qt_sb[0:64, hp, qs],
                    )
                    nc.tensor.matmul(
                        s01[:, QCH : 2 * QCH],
                        lhsT=kt_sb[64:128, hp, ks],
                        rhs=qt_sb[64:128, hp, qs],
                    )
                    e01 = ep.tile([128, 2 * QCH], b16, tag="e")
                    if kc >= 4 and kc % 2 == 0 and t < 4 * N_HP - 1:
                        # Schraudolph exp on VectorE (bf16 bits via int16)
                        nc.vector.tensor_scalar(
                            e01[:].bitcast(i16),
                            s01[:],
                            mbd_sb[:, kc : kc + 1],
                            0.0,
                            ADD,
                            MAX,
                        )
                    else:
                        # exact exp on ScalarE (scores pre-scaled by ALPHA)
                        nc.scalar.activation(
                            e01[:],
                            s01[:],
                            EXP,
                            bias=mba_sb[:, kc : kc + 1],
                            scale=1.0 / EXP_A,
                        )
                    pending.append((e01, kc))
                    if len(pending) > depth:
                        ctx_mm(pending.pop(0))
                for p in pending:
                    fill(2)
                    ctx_mm(p)

                # normalize: rows 64-127 of cc hold both heads' denominators
                # (replicated); relocate to base partition 0 (fast-reciprocal
                # breaks on shifted APs), one reciprocal, two multiplies.
                # Deferred into the next step's kc loop (see above) so the
                # serial chain doesn't head-of-line-block the DVE exps.
                def norm(hp=hp, qs=qs, cc=cc, c0=c0, c1=c1):
                    den01 = wp.tile([64, 2 * QCH], f32, tag="den", name="den")
                    nc.vector.tensor_copy(out=den01[:], in_=cc[64:128, :])
                    rc01 = wp.tile([64, 2 * QCH], f32, tag="rc", name="rc")
                    nc.vector.reciprocal_approx_fast(rc01[:], den01[:])
                    nc.vector.tensor_mul(
                        out=ctx_sb[0:64, hp, qs], in0=c0[0:64, :], in1=rc01[:, 0:QCH]
                    )
                    nc.vector.tensor_mul(
                        out=ctx_sb[64:128, hp, qs],
                        in0=c1[0:64, :],
                        in1=rc01[:, QCH : 2 * QCH],
                    )

                norm()

                # queue the q window needed a full qc ahead
                if qc < 3:
                    add_unit(
                        q_key(hp, qc + 1),
                        proj_qk_unit(
                            wq_sb, wq1_sb, qt_sb, hp, (qc + 1) * 512, 512, S
                        ),
                    )

            # drain: last q-chunk's out-projections (phase-split so the PE
            # stays warm across the final norm chain) + queue leftovers
            order = []
            for i in range(N_HP):
                rt = 3 * N_HP + i
                order.append(("opt1", rt))
                if i >= 1:
                    order.append(("opt2", rt - 1))
            order.append(("opt2", 3 * N_HP + N_HP - 1))
            for kind, rt in order:
                add_unit((kind, rt), op_tail_p1(rt) if kind == "opt1" else op_tail_p2(rt))
            while queue:
                fill(1)

    nc.finalize()
    return nc


def _pack(a, kc):
    """[kc*128, n] -> [128, kc*n] partition-major bf16 (SBUF layout)."""
    k128, n = a.shape
    return (
        np.ascontiguousarray(a.reshape(kc, 128, n).transpose(1, 0, 2))
        .reshape(128, kc * n)
        .astype(bf16)
    )


def _pack_w_hp(wT):
    """[D, DL] transposed weight -> [128, N_HP*KC8*128] hp-major."""
    a = wT.reshape(KC8, 128, N_HP, 128).transpose(1, 2, 0, 3)
    return np.ascontiguousarray(a).reshape(128, N_HP * KC8 * 128).astype(bf16)


def _host_prep(x, mask, wq, bq, wk, bk, wv, bv, wo):
    x = np.asarray(x, dtype=np.float32)
    mask = np.asarray(mask)
    idxs = [np.nonzero(mask[b])[0] for b in range(B)]
    nmax = max(1, max(len(i) for i in idxs))
    nkv = min(S, ((nmax + KV_P - 1) // KV_P) * KV_P)
    with_bias = bool(
        np.any(np.asarray(bq)) or np.any(np.asarray(bk)) or np.any(np.asarray(bv))
    )

    in_maps = []
    for c in range(DP * TP):
        b, g = c // TP, c % TP
        sl = slice(g * DL, g * DL + DL)

        idx = idxs[b]
        xg = np.zeros((nkv, D), dtype=np.float32)
        xg[: len(idx)] = x[b][idx]

        mba = np.full((nkv,), NEG, dtype=np.float32)
        mba[: len(idx)] = 0.0
        mbd = np.full((nkv,), NEG_DVE, dtype=np.float32)
        mbd[: len(idx)] = EXP_B

        im = {
            "xq": _pack(x[b].T, KC8),
            "xkv": _pack(xg.T, KC8),
            "wqt": _pack_w_hp(np.asarray(wq, dtype=np.float32)[sl, :].T * ALPHA),
            "wkt": _pack_w_hp(np.asarray(wk, dtype=np.float32)[sl, :].T),
            "wvt": _pack(np.asarray(wv)[sl, :].T, KC8),
            "wot": _pack(np.asarray(wo)[:, sl].T, DL // 128),
            "mbact": mba,
            "mbdve": mbd,
        }
        if with_bias:
            im["bq"] = (np.asarray(bq, dtype=np.float32)[None, sl] * ALPHA).astype(bf16)
            im["bk"] = np.asarray(bk)[None, sl].astype(bf16)
            im["bv"] = np.asarray(bv)[None, sl].astype(bf16)
        in_maps.append(im)
    return nkv, with_bias, in_maps


def kernel(x, mask, wq, bq, wk, bk, wv, bv, wo, bo):
    from concourse.bass_utils import run_bass_kernel_spmd

    nkv, with_bias, in_maps = _host_prep(x, mask, wq, bq, wk, bk, wv, bv, wo)
    nc = _build(nkv, with_bias)
    res = run_bass_kernel_spmd(nc, in_maps, core_ids=list(range(DP * TP)))

    out = np.empty((B, S, D), dtype=np.float32)
    bo = np.asarray(bo, dtype=np.float32)
    for b in range(B):
        out[b] = (
            res.results[b * TP]["out"].astype(np.float32)
            + res.results[b * TP + 1]["out"].astype(np.float32)
            + bo
        )
    return out


# revision 24
# speedup vs baseline: 1.0029x; 1.0029x over previous
"""Multi-head attention (B=4, S=2048, D=1024, H=16) on 8 TRN2 NeuronCores.

Sharding: DP=4 over batch x TP=2 over heads. Core c handles batch c//2 and
heads 8*(c%2) .. 8*(c%2)+8. Each core computes a partial output [S, D] (its
heads' contribution to the out-projection); the host sums the two TP partials
per batch and adds the output bias.

Key compaction: the key-padding mask removes ~half the keys, so the host
gathers unmasked key rows per batch (padded to a multiple of 128). k/v
projections and attention only touch NKV ~= S/2 keys; padding keys carry a
big negative additive bias fused into the exp so they contribute exactly 0.

On-chip layouts (all matmul operands bf16, accumulation fp32 in PSUM):
  qT/kT : [hd, seq] with the two heads of a pair stacked on partitions
          (0-63 / 64-127) -> the scores matmuls (K=64) auto-pack into PE
          row-groups and run concurrently.  qT is PRE-SCALED by
          ALPHA = SCALE * 128/ln2 so PSUM scores are in "bf16-bits" units.
  exp   : split across two engines so neither paces the kernel. ScalarE
          computes exact exp via its free affine (scale=1/A, bias=mask);
          for kc%3==2 VectorE computes a Schraudolph exp in ONE op:
          bits16 = max(scores + B, 0) converted to int16 and bit-viewed as
          bf16 (linear-in-log2 approx, ~1.8% rms sawtooth, softmax-scale
          invariant; masked keys land at exactly +0 -> exp 0).
  v_aug : [keys, v | ones(64)] -> the ctx matmul accumulates ctxT (rows 0-63)
          and the softmax denominator replicated across rows 64-127, so the
          normalization is a fast-reciprocal + multiply on VectorE.

Engine-queue discipline (queues are strict FIFO, so emission order is the
schedule): the exp-paced kc loop leaves ~0.5us/kc of PE slack, which is
filled by a global queue of deferred matmul chunks (remaining projections,
second v half, next q window, previous step's out-projection). force()
emits any still-queued unit a step depends on before that step's scores, so
FIFO deadlock is impossible. Input DMAs are issued from both the Sync and
GpSimd queues (each dma_start costs ~0.6us of issue time on its queue).
"""

import sys

sys.path.insert(0, "/opt/trn_rl_repo")

import numpy as np
import ml_dtypes

B, S, D, H = 4, 2048, 1024, 16
HD = D // H
SCALE = 1.0 / float(np.sqrt(HD))
NEG = -1e9

# Schraudolph exp-as-int16 constants (bf16 bit layout: 7 mantissa bits)
EXP_A = 128.0 / float(np.log(2.0))      # d bits / d ln(x)
ALPHA = EXP_A * SCALE                   # folded into wq on the host
EXP_B = 128.0 * (127.0 - 0.0573) + 0.5  # bits offset (softmax-invariant)
NEG_DVE = -1.0e6
USE_GP_DMA = True

DP = 4  # batch shards
TP = 2  # head-group shards
HL = H // TP  # heads per core (8)
DL = HL * HD  # local head dims per core (512)
N_HP = HL // 2  # head pairs per core (4)
QCH = 512  # q chunk (free dim of score matmuls)
KV_P = 128  # key chunk (partition dim of scoresT)
KC8 = D // 128  # contraction chunks for projections (8)

bf16 = ml_dtypes.bfloat16


def _windows(n, w=512):
    out = []
    off = 0
    while off < n:
        out.append((off, min(w, n - off)))
        off += w
    return out


def _build(nkv, with_bias=True):
    from concourse import bacc
    import concourse.mybir as mybir
    from concourse.tile import TileContext

    dt = mybir.dt
    f32 = dt.float32
    b16 = dt.bfloat16
    i16 = dt.int16
    EXP = mybir.ActivationFunctionType.Exp
    ADD = mybir.AluOpType.add
    MAX = mybir.AluOpType.max

    nkc = nkv // KV_P  # key chunks
    kwins = _windows(nkv)  # kt projection windows
    W1 = KC8 * 128  # weight cols per hp block

    nc = bacc.Bacc(trn_type="TRN2")

    xq_d = nc.dram_tensor("xq", (128, KC8 * S), b16, kind="ExternalInput").ap()
    xkv_d = nc.dram_tensor("xkv", (128, KC8 * nkv), b16, kind="ExternalInput").ap()
    wq_d = nc.dram_tensor("wqt", (128, N_HP * W1), b16, kind="ExternalInput").ap()
    wk_d = nc.dram_tensor("wkt", (128, N_HP * W1), b16, kind="ExternalInput").ap()
    wv_d = nc.dram_tensor("wvt", (128, KC8 * DL), b16, kind="ExternalInput").ap()
    if with_bias:
        bq_d = nc.dram_tensor("bq", (1, DL), b16, kind="ExternalInput").ap()
        bk_d = nc.dram_tensor("bk", (1, DL), b16, kind="ExternalInput").ap()
        bv_d = nc.dram_tensor("bv", (1, DL), b16, kind="ExternalInput").ap()
    wo_d = nc.dram_tensor("wot", (128, (DL // 128) * D), b16, kind="ExternalInput").ap()
    mba_d = nc.dram_tensor("mbact", (nkv,), f32, kind="ExternalInput").ap()
    mbd_d = nc.dram_tensor("mbdve", (nkv,), f32, kind="ExternalInput").ap()
    out_d = nc.dram_tensor("out", (S, D), b16, kind="ExternalOutput").ap()

    with TileContext(nc) as tc:
        with (
            tc.tile_pool(name="persist", bufs=1) as pp,
            tc.tile_pool(name="ps_s", bufs=2, space="PSUM") as ps_s,
            tc.tile_pool(name="ps_cc", bufs=1, space="PSUM") as ps_cc,
            tc.tile_pool(name="ps_aux", bufs=2, space="PSUM") as ps_aux,
            tc.tile_pool(name="etile", bufs=12) as ep,
            tc.tile_pool(name="work", bufs=2) as wp,
            tc.tile_pool(name="ob", bufs=3) as obp,
        ):
            # ---- persistent SBUF tensors ----
            xq_sb = pp.tile([128, KC8, S], b16, tag="xq")
            xq1_sb = pp.tile([1, S], b16, tag="xq1")
            xkv_sb = pp.tile([128, KC8, nkv], b16, tag="xkv")
            xkv1_sb = pp.tile([1, nkv], b16, tag="xkv1")
            wq_sb = pp.tile([128, N_HP, KC8, 128], b16, tag="wq")
            wk_sb = pp.tile([128, N_HP, KC8, 128], b16, tag="wk")
            wv_sb = pp.tile([128, KC8, DL], b16, tag="wv")
            wq1_sb = pp.tile([1, DL], b16, tag="wq1")
            wk1_sb = pp.tile([1, DL], b16, tag="wk1")
            wv1_sb = pp.tile([1, DL], b16, tag="wv1")
            wo_sb = pp.tile([128, DL // 128, D], b16, tag="wo")
            mba_sb = pp.tile([128, nkc], f32, tag="mba")
            mbd_sb = pp.tile([128, nkc], f32, tag="mbd")
            qt_sb = pp.tile([128, N_HP, S], b16, tag="qt")
            kt_sb = pp.tile([128, N_HP, nkv], b16, tag="kt")
            v_sb = pp.tile([128, nkc, HL, 128], b16, tag="v")
            ctx_sb = pp.tile([128, N_HP, S], b16, tag="ctx")

            # ---- DMA staging, split across the Sync and GpSimd issue
            # queues; ordered so the first score matmuls gate on ~2.5MB ----
            gp = nc.gpsimd if USE_GP_DMA else nc.sync

            def load_w(eng, dst, src, hp):
                eng.dma_start(
                    dst[:, hp].rearrange("p kc e -> p (kc e)"),
                    src[:, hp * W1 : hp * W1 + W1],
                )

            def load_x_win(eng, dst, src, tot, off, n):
                for kc in range(KC8):
                    eng.dma_start(
                        dst[:, kc, off : off + n],
                        src[:, kc * tot + off : kc * tot + off + n],
                    )

            nc.sync.dma_start(
                wq_sb[:, 0].rearrange("p kc e -> p (kc e)"), wq_d[:, 0:W1]
            )
            load_w(gp, wk_sb, wk_d, 0)
            load_x_win(nc.sync, xq_sb, xq_d, S, 0, 512)
            load_x_win(gp, xkv_sb, xkv_d, nkv, 0, min(512, nkv))
            gp.dma_start(mba_sb[:], mba_d.rearrange("(kc p) -> p kc", p=128))
            gp.dma_start(mbd_sb[:], mbd_d.rearrange("(kc p) -> p kc", p=128))
            if with_bias:
                gp.dma_start(wq1_sb[:], bq_d)
                gp.dma_start(wk1_sb[:], bk_d)
                gp.dma_start(wv1_sb[:], bv_d)
            for kc in range(KC8):  # v-proj weights
                nc.sync.dma_start(wv_sb[:, kc, :], wv_d[:, kc * DL : kc * DL + DL])
            for off, n in kwins[1:]:
                load_x_win(gp, xkv_sb, xkv_d, nkv, off, n)
            for kc in range(DL // 128):  # wo needed by t=1
                nc.sync.dma_start(wo_sb[:, kc, :], wo_d[:, kc * D : kc * D + D])
            for hp in range(1, N_HP):
                load_w(gp, wk_sb, wk_d, hp)
                load_w(gp, wq_sb, wq_d, hp)
            for off, n in _windows(S)[1:]:
                load_x_win(nc.sync, xq_sb, xq_d, S, off, n)

            # constants
            if with_bias:
                nc.vector.memset(xq1_sb[:], 1.0)
                nc.vector.memset(xkv1_sb[:], 1.0)
            nc.vector.memset(v_sb[:, :, :, 64:128], 1.0)

            # ================= deferred-unit machinery =================
            # unit = list of chunk thunks (each ~0.4-0.9us of PE work);
            # fill(n) emits n chunks from the queue head; force(key) emits
            # a whole unit immediately (dependency safety).
            units = {}     # key -> list of remaining thunks
            queue = []     # ordered keys
            started = set()  # units with some chunks already emitted (their
            # PSUM aux tile is live; never interleave another unit's chunks
            # before they finish)

            def add_unit(key, thunks, front=False):
                units[key] = list(thunks)
                if front:
                    pos = 1 if (queue and queue[0] in started) else 0
                    queue.insert(pos, key)
                else:
                    queue.append(key)

            def fill(n):
                done = 0
                while done < n and queue:
                    key = queue[0]
                    th = units[key]
                    if th:
                        th.pop(0)()
                        started.add(key)
                        done += 1
                    if not th:
                        queue.pop(0)
                        started.discard(key)
                        del units[key]

            def force(key):
                if key in units:
                    for th in units[key]:
                        th()
                    units[key].clear()
                    if key in queue:
                        queue.remove(key)
                    started.discard(key)
                    del units[key]

            # ---- projection unit builders ----
            def proj_qk_unit(w_sb, w1_sb, dst_sb, hp, off, n, nfree):
                """chunks computing dst[:, hp, off:off+n]"""
                x_sb, x1_sb = (xq_sb, xq1_sb) if nfree == S else (xkv_sb, xkv1_sb)
                state = {}

                def mm2(kc0):
                    def th():
                        if "ps" not in state:
                            state["ps"] = ps_aux.tile([128, 512], f32, tag="aux", name="auxps")
                        ps = state["ps"]
                        for kc in (kc0, kc0 + 1):
                            nc.tensor.matmul(
                                ps[:, :n],
                                lhsT=w_sb[:, hp, kc, :],
                                rhs=x_sb[:, kc, off : off + n],
                                start=(kc == 0),
                                stop=(not with_bias and kc == KC8 - 1),
                            )

                    return th

                def tail():
                    ps = state["ps"]
                    if with_bias:
                        nc.tensor.matmul(
                            ps[:, :n],
                            lhsT=w1_sb[:, hp * 128 : hp * 128 + 128],
                            rhs=x1_sb[:, off : off + n],
                            start=False,
                            stop=True,
                        )
                    nc.scalar.copy(out=dst_sb[:, hp, off : off + n], in_=ps[:, :n])

                return [mm2(0), mm2(2), mm2(4), lambda: (mm2(6)(), tail())]

            def v_unit(mt, half):
                """v[keys mt*128:+128, heads half*4..+4] into v_sb."""
                hs = slice(half * 256, half * 256 + 256)
                state = {}

                def mm4(kc0):
                    def th():
                        if "ps" not in state:
                            state["ps"] = ps_aux.tile([128, 512], f32, tag="aux", name="auxps")
                        ps = state["ps"]
                        for kc in range(kc0, kc0 + 4):
                            nc.tensor.matmul(
                                ps[:, 0:256],
                                lhsT=xkv_sb[:, kc, mt * 128 : mt * 128 + 128],
                                rhs=wv_sb[:, kc, hs],
                                start=(kc == 0),
                                stop=(not with_bias and kc == KC8 - 1),
                            )

                    return th

                def tail():
                    ps = state["ps"]
                    if with_bias:
                        nc.tensor.matmul(
                            ps[:, 0:256],
                            lhsT=xkv1_sb[:, mt * 128 : mt * 128 + 128],
                            rhs=wv1_sb[:, hs],
                            start=False,
                            stop=True,
                        )
                    nc.scalar.copy(
                        out=v_sb[:, mt, half * 4 : half * 4 + 4, 0:64],
                        in_=ps[:, 0:256].rearrange("p (h e) -> p h e", h=4),
                    )

                return [mm4(0), lambda: (mm4(4)(), tail())]

            def op_unit(rt):
                """out-projection for row-tile rt: 2 psum halves + copies."""
                rs = slice(rt * 128, rt * 128 + 128)
                state = {}

                def mm2(nj, k0):
                    def th():
                        key = f"ps{nj}"
                        if key not in state:
                            state[key] = ps_aux.tile([128, 512], f32, tag="aux", name="auxps")
                        ps = state[key]
                        ns = slice(nj * 512, nj * 512 + 512)
                        for khp in (k0, k0 + 1):
                            nc.tensor.matmul(
                                ps[:],
                                lhsT=ctx_sb[:, khp, rs],
                                rhs=wo_sb[:, khp, ns],
                                start=(khp == 0),
                                stop=(khp == N_HP - 1),
                            )

                    return th

                def copy(nj):
                    if "ob" not in state:
                        state["ob"] = obp.tile([128, D], b16, tag="ob", name="obt")
                    nc.scalar.copy(
                        out=state["ob"][:, nj * 512 : nj * 512 + 512],
                        in_=state[f"ps{nj}"][:],
                    )

                def tail():
                    mm2(1, 2)()
                    copy(1)
                    nc.sync.dma_start(out_d[rs, :], state["ob"][:])

                # khp 0,1 chunks first: the khp 2,3 halves wait on the
                # previous step's norm, which runs at the start of this
                # step on the otherwise-idle VectorE
                return [
                    mm2(0, 0),
                    mm2(1, 0),
                    lambda: (mm2(0, 2)(), copy(0)),
                    tail,
                ]

            # tail out-projection for the last q-chunk, split in two phases:
            # phase 1 (khp 0,1) depends only on earlier steps and keeps the
            # PE warm while the last norm chain runs; phase 2 (khp 2,3 +
            # copy + DMA) waits on the final normalizations.  PSUM comes
            # from the s-pool ([128,1024] tiles, free once exps are done).
            op_tail_state = {}

            def op_tail_p1(rt):
                rs = slice(rt * 128, rt * 128 + 128)

                def th():
                    ps = ps_s.tile([128, D], f32, tag="s", name="opt")
                    op_tail_state[rt] = ps
                    for nj in range(D // 512):
                        ns = slice(nj * 512, nj * 512 + 512)
                        for khp in (0, 1):
                            nc.tensor.matmul(
                                ps[:, ns],
                                lhsT=ctx_sb[:, khp, rs],
                                rhs=wo_sb[:, khp, ns],
                                start=(khp == 0),
                                stop=False,
                            )

                return [th]

            def op_tail_p2(rt):
                rs = slice(rt * 128, rt * 128 + 128)

                def th():
                    ps = op_tail_state[rt]
                    for nj in range(D // 512):
                        ns = slice(nj * 512, nj * 512 + 512)
                        for khp in (2, 3):
                            nc.tensor.matmul(
                                ps[:, ns],
                                lhsT=ctx_sb[:, khp, rs],
                                rhs=wo_sb[:, khp, ns],
                                start=False,
                                stop=(khp == N_HP - 1),
                            )
                    ob = obp.tile([128, D], b16, tag="ob", name="obt")
                    if rt % 2 == 0:
                        nc.scalar.copy(out=ob[:], in_=ps[:])
                    else:
                        nc.vector.tensor_copy(out=ob[:], in_=ps[:])
                    nc.sync.dma_start(out_d[rs, :], ob[:])

                return [th]

            def q_key(hp, qc):
                return ("q", hp, qc)

            def k_key(hp, w):
                return ("k", hp, w)

            def v_key(mt, half):
                return ("v", mt, half)

            # prologue: only what gates the first exp
            force_emit = proj_qk_unit(wq_sb, wq1_sb, qt_sb, 0, 0, 512, S)
            for th in force_emit:
                th()
            kw0 = proj_qk_unit(wk_sb, wk1_sb, kt_sb, 0, 0, kwins[0][1], nkv)
            for th in kw0:
                th()

            # queue: rest of kt hp0, v half0, then hp1.. (force() is the net)
            for wi, (off, n) in enumerate(kwins[1:], start=1):
                add_unit(k_key(0, wi), proj_qk_unit(wk_sb, wk1_sb, kt_sb, 0, off, n, nkv))
            for mt in range(nkc):
                add_unit(v_key(mt, 0), v_unit(mt, 0))
            for hp in range(1, N_HP):
                add_unit(q_key(hp, 0), proj_qk_unit(wq_sb, wq1_sb, qt_sb, hp, 0, 512, S))
                for wi, (off, n) in enumerate(kwins):
                    add_unit(
                        k_key(hp, wi),
                        proj_qk_unit(wk_sb, wk1_sb, kt_sb, hp, off, n, nkv),
                    )
                if hp == 1:
                    for mt in range(nkc):
                        add_unit(v_key(mt, 1), v_unit(mt, 1))

            # ================= attention steps =================
            for t in range(4 * N_HP):
                qc, hp = divmod(t, N_HP)
                qs = slice(qc * QCH, qc * QCH + QCH)
                half = hp // 2

                # dependency safety: everything this step reads must be
                # emitted before its consumers
                force(q_key(hp, qc))
                for wi in range(len(kwins)):
                    force(k_key(hp, wi))
                for mt in range(nkc):
                    force(v_key(mt, half))

                # out-projection for row-tile (qc-1, hp): all head-pairs of
                # q-chunk qc-1 are complete by now; its chunks go to the
                # queue front to fill this step's kc-loop PE slack
                if qc > 0:
                    rt = (qc - 1) * N_HP + hp
                    add_unit(("op", rt), op_unit(rt), front=True)

                cc = ps_cc.tile([128, 2 * QCH], f32, tag="cc")
                c0 = cc[:, 0:QCH]
                c1 = cc[:, QCH : 2 * QCH]

                def ctx_mm(ekc, hp=hp, c0=c0, c1=c1):
                    e01_p, kc_p = ekc
                    nc.tensor.matmul(
                        c0,
                        lhsT=v_sb[:, kc_p, 2 * hp, :],
                        rhs=e01_p[:, 0:QCH],
                        start=(kc_p == 0),
                        stop=(kc_p == nkc - 1),
                    )
                    nc.tensor.matmul(
                        c1,
                        lhsT=v_sb[:, kc_p, 2 * hp + 1, :],
                        rhs=e01_p[:, QCH : 2 * QCH],
                        start=(kc_p == 0),
                        stop=(kc_p == nkc - 1),
                    )

                depth = nkc if t == 0 else 4
                pending = []
                for kc in range(nkc):
                    ks = slice(kc * KV_P, kc * KV_P + KV_P)
                    fill(1)
                    s01 = ps_s.tile([128, 2 * QCH], f32, tag="s")
                    nc.tensor.matmul(
                        s01[:, 0:QCH],
                        lhsT=kt_sb[0:64, hp, ks],
                        rhs=qt_sb[0:64, hp, qs],
                    )
                    nc.tensor.matmul(
                        s01[:, QCH : 2 * QCH],
                        lhsT=kt_sb[64:128, hp, ks],
                        rhs=qt_sb[64:128, hp, qs],
                    )
                    e01 = ep.tile([128, 2 * QCH], b16, tag="e")
                    if kc >= 4 and kc % 2 == 0 and t < 4 * N_HP - 1:
                        # Schraudolph exp on VectorE (bf16 bits via int16)
                        nc.vector.tensor_scalar(
                            e01[:].bitcast(i16),
                            s01[:],
                            mbd_sb[:, kc : kc + 1],
                            0.0,
                            ADD,
                            MAX,
                        )
                    else:
                        # exact exp on ScalarE (scores pre-scaled by ALPHA)
                        nc.scalar.activation(
                            e01[:],
                            s01[:],
                            EXP,
                            bias=mba_sb[:, kc : kc + 1],
                            scale=1.0 / EXP_A,
                        )
                    pending.append((e01, kc))
                    if len(pending) > depth:
                        ctx_mm(pending.pop(0))
                for p in pending:
                    fill(2)
                    ctx_mm(p)

                # normalize: rows 64-127 of cc hold both heads' denominators
                # (replicated); relocate to base partition 0 (fast-reciprocal
                # breaks on shifted APs), one reciprocal, two multiplies.
                # Deferred into the next step's kc loop (see above) so the
                # serial chain doesn't head-of-line-block the DVE exps.
                def norm(hp=hp, qs=qs, cc=cc, c0=c0, c1=c1):
                    den01 = wp.tile([64, 2 * QCH], f32, tag="den", name="den")
                    nc.vector.tensor_copy(out=den01[:], in_=cc[64:128, :])
                    rc01 = wp.tile([64, 2 * QCH], f32, tag="rc", name="rc")
                    nc.vector.reciprocal_approx_fast(rc01[:], den01[:])
                    nc.vector.tensor_mul(
                        out=ctx_sb[0:64, hp, qs], in0=c0[0:64, :], in1=rc01[:, 0:QCH]
                    )
                    nc.vector.tensor_mul(
                        out=ctx_sb[64:128, hp, qs],
                        in0=c1[0:64, :],
                        in1=rc01[:, QCH : 2 * QCH],
                    )

                norm()

                # queue the q window needed a full qc ahead
                if qc < 3:
                    add_unit(
                        q_key(hp, qc + 1),
                        proj_qk_unit(
                            wq_sb, wq1_sb, qt_sb, hp, (qc + 1) * 512, 512, S
                        ),
                    )

            # drain: last q-chunk's out-projections (phase-split so the PE
            # stays warm across the final norm chain) + queue leftovers
            order = []
            for i in range(N_HP):
                rt = 3 * N_HP + i
                order.append(("opt1", rt))
                if i >= 1:
                    order.append(("opt2", rt - 1))
            order.append(("opt2", 3 * N_HP + N_HP - 1))
            for kind, rt in order:
                add_unit((kind, rt), op_tail_p1(rt) if kind == "opt1" else op_tail_p2(rt))
            while queue:
                fill(1)

    nc.finalize()
    return nc


def _pack(a, kc):
    """[kc*128, n] -> [128, kc*n] partition-major bf16 (SBUF layout)."""
    k128, n = a.shape
    return (
        np.ascontiguousarray(a.reshape(kc, 128, n).transpose(1, 0, 2))
        .reshape(128, kc * n)
        .astype(bf16)
    )


def _pack_w_hp(wT):
    """[D, DL] transposed weight -> [128, N_HP*KC8*128] hp-major."""
    a = wT.reshape(KC8, 128, N_HP, 128).transpose(1, 2, 0, 3)
    return np.ascontiguousarray(a).reshape(128, N_HP * KC8 * 128).astype(bf16)


def _host_prep(x, mask, wq, bq, wk, bk, wv, bv, wo):
    x = np.asarray(x, dtype=np.float32)
    mask = np.asarray(mask)
    idxs = [np.nonzero(mask[b])[0] for b in range(B)]
    nmax = max(1, max(len(i) for i in idxs))
    nkv = min(S, ((nmax + KV_P - 1) // KV_P) * KV_P)
    with_bias = bool(
        np.any(np.asarray(bq)) or np.any(np.asarray(bk)) or np.any(np.asarray(bv))
    )

    in_maps = []
    for c in range(DP * TP):
        b, g = c // TP, c % TP
        sl = slice(g * DL, g * DL + DL)

        idx = idxs[b]
        xg = np.zeros((nkv, D), dtype=np.float32)
        xg[: len(idx)] = x[b][idx]

        mba = np.full((nkv,), NEG, dtype=np.float32)
        mba[: len(idx)] = 0.0
        mbd = np.full((nkv,), NEG_DVE, dtype=np.float32)
        mbd[: len(idx)] = EXP_B

        im = {
            "xq": _pack(x[b].T, KC8),
            "xkv": _pack(xg.T, KC8),
            "wqt": _pack_w_hp(np.asarray(wq, dtype=np.float32)[sl, :].T * ALPHA),
            "wkt": _pack_w_hp(np.asarray(wk, dtype=np.float32)[sl, :].T),
            "wvt": _pack(np.asarray(wv)[sl, :].T, KC8),
            "wot": _pack(np.asarray(wo)[:, sl].T, DL // 128),
            "mbact": mba,
            "mbdve": mbd,
        }
        if with_bias:
            im["bq"] = (np.asarray(bq, dtype=np.float32)[None, sl] * ALPHA).astype(bf16)
            im["bk"] = np.asarray(bk)[None, sl].astype(bf16)
            im["bv"] = np.asarray(bv)[None, sl].astype(bf16)
        in_maps.append(im)
    return nkv, with_bias, in_maps


def kernel(x, mask, wq, bq, wk, bk, wv, bv, wo, bo):
    from concourse.bass_utils import run_bass_kernel_spmd

    nkv, with_bias, in_maps = _host_prep(x, mask, wq, bq, wk, bk, wv, bv, wo)
    nc = _build(nkv, with_bias)
    res = run_bass_kernel_spmd(nc, in_maps, core_ids=list(range(DP * TP)))

    out = np.empty((B, S, D), dtype=np.float32)
    bo = np.asarray(bo, dtype=np.float32)
    for b in range(B):
        out[b] = (
            res.results[b * TP]["out"].astype(np.float32)
            + res.results[b * TP + 1]["out"].astype(np.float32)
            + bo
        )
    return out


# revision 27
# speedup vs baseline: 1.0193x; 1.0164x over previous
"""Multi-head attention (B=4, S=2048, D=1024, H=16) on 8 TRN2 NeuronCores.

Sharding: DP=4 over batch x TP=2 over heads. Core c handles batch c//2 and
heads 8*(c%2) .. 8*(c%2)+8. Each core computes a partial output [S, D] (its
heads' contribution to the out-projection); the host sums the two TP partials
per batch and adds the output bias.

Key compaction: the key-padding mask removes ~half the keys, so the host
gathers unmasked key rows per batch (padded to a multiple of 128). k/v
projections and attention only touch NKV ~= S/2 keys; padding keys carry a
big negative additive bias fused into the exp so they contribute exactly 0.

On-chip layouts (all matmul operands bf16, accumulation fp32 in PSUM):
  qT/kT : [hd, seq] with the two heads of a pair stacked on partitions
          (0-63 / 64-127) -> the scores matmuls (K=64) auto-pack into PE
          row-groups and run concurrently.  qT is PRE-SCALED by
          ALPHA = SCALE * 128/ln2 so PSUM scores are in "bf16-bits" units.
  exp   : split across two engines so neither paces the kernel. ScalarE
          computes exact exp via its free affine (scale=1/A, bias=mask);
          for kc%3==2 VectorE computes a Schraudolph exp in ONE op:
          bits16 = max(scores + B, 0) converted to int16 and bit-viewed as
          bf16 (linear-in-log2 approx, ~1.8% rms sawtooth, softmax-scale
          invariant; masked keys land at exactly +0 -> exp 0).
  v_aug : [keys, v | ones(64)] -> the ctx matmul accumulates ctxT (rows 0-63)
          and the softmax denominator replicated across rows 64-127, so the
          normalization is a fast-reciprocal + multiply on VectorE.

Engine-queue discipline (queues are strict FIFO, so emission order is the
schedule): the exp-paced kc loop leaves ~0.5us/kc of PE slack, which is
filled by a global queue of deferred matmul chunks (remaining projections,
second v half, next q window, previous step's out-projection). force()
emits any still-queued unit a step depends on before that step's scores, so
FIFO deadlock is impossible. Input DMAs are issued from both the Sync and
GpSimd queues (each dma_start costs ~0.6us of issue time on its queue).
"""

import sys

sys.path.insert(0, "/opt/trn_rl_repo")

import numpy as np
import ml_dtypes

B, S, D, H = 4, 2048, 1024, 16
HD = D // H
SCALE = 1.0 / float(np.sqrt(HD))
NEG = -1e9

# Schraudolph exp-as-int16 constants (bf16 bit layout: 7 mantissa bits)
EXP_A = 128.0 / float(np.log(2.0))      # d bits / d ln(x)
ALPHA = EXP_A * SCALE                   # folded into wq on the host
EXP_B = 128.0 * (127.0 - 0.0573) + 0.5  # bits offset (softmax-invariant)
NEG_DVE = -1.0e6
USE_GP_DMA = True

DP = 4  # batch shards
TP = 2  # head-group shards
HL = H // TP  # heads per core (8)
DL = HL * HD  # local head dims per core (512)
N_HP = HL // 2  # head pairs per core (4)
QCH = 512  # q chunk (free dim of score matmuls)
KV_P = 128  # key chunk (partition dim of scoresT)
KC8 = D // 128  # contraction chunks for projections (8)

bf16 = ml_dtypes.bfloat16


def _windows(n, w=512):
    out = []
    off = 0
    while off < n:
        out.append((off, min(w, n - off)))
        off += w
    return out


def _build(nkv, with_bias=True):
    from concourse import bacc
    import concourse.mybir as mybir
    from concourse.tile import TileContext

    dt = mybir.dt
    f32 = dt.float32
    b16 = dt.bfloat16
    i16 = dt.int16
    EXP = mybir.ActivationFunctionType.Exp
    ADD = mybir.AluOpType.add
    MAX = mybir.AluOpType.max

    nkc = nkv // KV_P  # key chunks
    kwins = _windows(nkv)  # kt projection windows
    W1 = KC8 * 128  # weight cols per hp block

    nc = bacc.Bacc(trn_type="TRN2")

    xq_d = nc.dram_tensor("xq", (128, KC8 * S), b16, kind="ExternalInput").ap()
    xkv_d = nc.dram_tensor("xkv", (128, KC8 * nkv), b16, kind="ExternalInput").ap()
    wq_d = nc.dram_tensor("wqt", (128, N_HP * W1), b16, kind="ExternalInput").ap()
    wk_d = nc.dram_tensor("wkt", (128, N_HP * W1), b16, kind="ExternalInput").ap()
    wv_d = nc.dram_tensor("wvt", (128, KC8 * DL), b16, kind="ExternalInput").ap()
    if with_bias:
        bq_d = nc.dram_tensor("bq", (1, DL), b16, kind="ExternalInput").ap()
        bk_d = nc.dram_tensor("bk", (1, DL), b16, kind="ExternalInput").ap()
        bv_d = nc.dram_tensor("bv", (1, DL), b16, kind="ExternalInput").ap()
    wo_d = nc.dram_tensor("wot", (128, (DL // 128) * D), b16, kind="ExternalInput").ap()
    mba_d = nc.dram_tensor("mbact", (nkv,), f32, kind="ExternalInput").ap()
    mbd_d = nc.dram_tensor("mbdve", (nkv,), f32, kind="ExternalInput").ap()
    out_d = nc.dram_tensor("out", (S, D), b16, kind="ExternalOutput").ap()

    with TileContext(nc) as tc:
        with (
            tc.tile_pool(name="persist", bufs=1) as pp,
            tc.tile_pool(name="ps_s", bufs=2, space="PSUM") as ps_s,
            tc.tile_pool(name="ps_cc", bufs=1, space="PSUM") as ps_cc,
            tc.tile_pool(name="ps_aux", bufs=2, space="PSUM") as ps_aux,
            tc.tile_pool(name="etile", bufs=12) as ep,
            tc.tile_pool(name="work", bufs=2) as wp,
            tc.tile_pool(name="ob", bufs=3) as obp,
        ):
            # ---- persistent SBUF tensors ----
            xq_sb = pp.tile([128, KC8, S], b16, tag="xq")
            xq1_sb = pp.tile([1, S], b16, tag="xq1")
            xkv_sb = pp.tile([128, KC8, nkv], b16, tag="xkv")
            xkv1_sb = pp.tile([1, nkv], b16, tag="xkv1")
            wq_sb = pp.tile([128, N_HP, KC8, 128], b16, tag="wq")
            wk_sb = pp.tile([128, N_HP, KC8, 128], b16, tag="wk")
            wv_sb = pp.tile([128, KC8, DL], b16, tag="wv")
            wq1_sb = pp.tile([1, DL], b16, tag="wq1")
            wk1_sb = pp.tile([1, DL], b16, tag="wk1")
            wv1_sb = pp.tile([1, DL], b16, tag="wv1")
            wo_sb = pp.tile([128, DL // 128, D], b16, tag="wo")
            mba_sb = pp.tile([128, nkc], f32, tag="mba")
            mbd_sb = pp.tile([128, nkc], f32, tag="mbd")
            qt_sb = pp.tile([128, N_HP, S], b16, tag="qt")
            kt_sb = pp.tile([128, N_HP, nkv], b16, tag="kt")
            v_sb = pp.tile([128, nkc, HL, 128], b16, tag="v")
            ctx_sb = pp.tile([128, N_HP, S], b16, tag="ctx")

            # ---- DMA staging, split across the Sync and GpSimd issue
            # queues; ordered so the first score matmuls gate on ~2.5MB ----
            gp = nc.gpsimd if USE_GP_DMA else nc.sync

            def load_w(eng, dst, src, hp):
                eng.dma_start(
                    dst[:, hp].rearrange("p kc e -> p (kc e)"),
                    src[:, hp * W1 : hp * W1 + W1],
                )

            def load_x_win(eng, dst, src, tot, off, n):
                for kc in range(KC8):
                    eng.dma_start(
                        dst[:, kc, off : off + n],
                        src[:, kc * tot + off : kc * tot + off + n],
                    )

            nc.sync.dma_start(
                wq_sb[:, 0].rearrange("p kc e -> p (kc e)"), wq_d[:, 0:W1]
            )
            load_w(gp, wk_sb, wk_d, 0)
            load_x_win(nc.sync, xq_sb, xq_d, S, 0, 512)
            load_x_win(gp, xkv_sb, xkv_d, nkv, 0, min(512, nkv))
            gp.dma_start(mba_sb[:], mba_d.rearrange("(kc p) -> p kc", p=128))
            gp.dma_start(mbd_sb[:], mbd_d.rearrange("(kc p) -> p kc", p=128))
            if with_bias:
                gp.dma_start(wq1_sb[:], bq_d)
                gp.dma_start(wk1_sb[:], bk_d)
                gp.dma_start(wv1_sb[:], bv_d)
            for kc in range(KC8):  # v-proj weights
                nc.sync.dma_start(wv_sb[:, kc, :], wv_d[:, kc * DL : kc * DL + DL])
            for off, n in kwins[1:]:
                load_x_win(gp, xkv_sb, xkv_d, nkv, off, n)
            for kc in range(DL // 128):  # wo needed by t=1
                nc.sync.dma_start(wo_sb[:, kc, :], wo_d[:, kc * D : kc * D + D])
            for hp in range(1, N_HP):
                load_w(gp, wk_sb, wk_d, hp)
                load_w(gp, wq_sb, wq_d, hp)
            for off, n in _windows(S)[1:]:
                load_x_win(nc.sync, xq_sb, xq_d, S, off, n)

            # constants
            if with_bias:
                nc.vector.memset(xq1_sb[:], 1.0)
                nc.vector.memset(xkv1_sb[:], 1.0)
            nc.vector.memset(v_sb[:, :, :, 64:128], 1.0)

            # ================= deferred-unit machinery =================
            # unit = list of chunk thunks (each ~0.4-0.9us of PE work);
            # fill(n) emits n chunks from the queue head; force(key) emits
            # a whole unit immediately (dependency safety).
            units = {}     # key -> list of remaining thunks
            queue = []     # ordered keys
            started = set()  # units with some chunks already emitted (their
            # PSUM aux tile is live; never interleave another unit's chunks
            # before they finish)

            def add_unit(key, thunks, front=False):
                units[key] = list(thunks)
                if front:
                    pos = 1 if (queue and queue[0] in started) else 0
                    queue.insert(pos, key)
                else:
                    queue.append(key)

            def fill(n):
                done = 0
                while done < n and queue:
                    key = queue[0]
                    th = units[key]
                    if th:
                        th.pop(0)()
                        started.add(key)
                        done += 1
                    if not th:
                        queue.pop(0)
                        started.discard(key)
                        del units[key]

            def force(key):
                if key in units:
                    for th in units[key]:
                        th()
                    units[key].clear()
                    if key in queue:
                        queue.remove(key)
                    started.discard(key)
                    del units[key]

            # ---- projection unit builders ----
            def proj_qk_unit(w_sb, w1_sb, dst_sb, hp, off, n, nfree):
                """chunks computing dst[:, hp, off:off+n]"""
                x_sb, x1_sb = (xq_sb, xq1_sb) if nfree == S else (xkv_sb, xkv1_sb)
                state = {}

                def mm2(kc0):
                    def th():
                        if "ps" not in state:
                            state["ps"] = ps_aux.tile([128, 512], f32, tag="aux", name="auxps")
                        ps = state["ps"]
                        for kc in (kc0, kc0 + 1):
                            nc.tensor.matmul(
                                ps[:, :n],
                                lhsT=w_sb[:, hp, kc, :],
                                rhs=x_sb[:, kc, off : off + n],
                                start=(kc == 0),
                                stop=(not with_bias and kc == KC8 - 1),
                            )

                    return th

                def tail():
                    ps = state["ps"]
                    if with_bias:
                        nc.tensor.matmul(
                            ps[:, :n],
                            lhsT=w1_sb[:, hp * 128 : hp * 128 + 128],
                            rhs=x1_sb[:, off : off + n],
                            start=False,
                            stop=True,
                        )
                    nc.scalar.copy(out=dst_sb[:, hp, off : off + n], in_=ps[:, :n])

                return [mm2(0), mm2(2), mm2(4), lambda: (mm2(6)(), tail())]

            def v_unit(mt, half):
                """v[keys mt*128:+128, heads half*4..+4] into v_sb."""
                hs = slice(half * 256, half * 256 + 256)
                state = {}

                def mm4(kc0):
                    def th():
                        if "ps" not in state:
                            state["ps"] = ps_aux.tile([128, 512], f32, tag="aux", name="auxps")
                        ps = state["ps"]
                        for kc in range(kc0, kc0 + 4):
                            nc.tensor.matmul(
                                ps[:, 0:256],
                                lhsT=xkv_sb[:, kc, mt * 128 : mt * 128 + 128],
                                rhs=wv_sb[:, kc, hs],
                                start=(kc == 0),
                                stop=(not with_bias and kc == KC8 - 1),
                            )

                    return th

                def tail():
                    ps = state["ps"]
                    if with_bias:
                        nc.tensor.matmul(
                            ps[:, 0:256],
                            lhsT=xkv1_sb[:, mt * 128 : mt * 128 + 128],
                            rhs=wv1_sb[:, hs],
                            start=False,
                            stop=True,
                        )
                    nc.scalar.copy(
                        out=v_sb[:, mt, half * 4 : half * 4 + 4, 0:64],
                        in_=ps[:, 0:256].rearrange("p (h e) -> p h e", h=4),
                    )

                return [mm4(0), lambda: (mm4(4)(), tail())]

            def op_unit(rt):
                """out-projection for row-tile rt: 2 psum halves + copies."""
                rs = slice(rt * 128, rt * 128 + 128)
                state = {}

                def mm2(nj, k0):
                    def th():
                        key = f"ps{nj}"
                        if key not in state:
                            state[key] = ps_aux.tile([128, 512], f32, tag="aux", name="auxps")
                        ps = state[key]
                        ns = slice(nj * 512, nj * 512 + 512)
                        for khp in (k0, k0 + 1):
                            nc.tensor.matmul(
                                ps[:],
                                lhsT=ctx_sb[:, khp, rs],
                                rhs=wo_sb[:, khp, ns],
                                start=(khp == 0),
                                stop=(khp == N_HP - 1),
                            )

                    return th

                def copy(nj):
                    if "ob" not in state:
                        state["ob"] = obp.tile([128, D], b16, tag="ob", name="obt")
                    nc.scalar.copy(
                        out=state["ob"][:, nj * 512 : nj * 512 + 512],
                        in_=state[f"ps{nj}"][:],
                    )

                def tail():
                    mm2(1, 2)()
                    copy(1)
                    nc.sync.dma_start(out_d[rs, :], state["ob"][:])

                # four explicitly-placeable thunks: A-chunks (khp 0,1) are
                # ready immediately; B-chunks (khp 2,3) wait on the previous
                # step's norm, which runs kc0-3 on VectorE -- the step
                # scheduler places them at kc>=5
                return {
                    "a0": mm2(0, 0),
                    "a1": mm2(1, 0),
                    "b0": lambda: (mm2(0, 2)(), copy(0)),
                    "b1": tail,
                }

            # tail out-projection for the last q-chunk, split in two phases:
            # phase 1 (khp 0,1) depends only on earlier steps and keeps the
            # PE warm while the last norm chain runs; phase 2 (khp 2,3 +
            # copy + DMA) waits on the final normalizations.  PSUM comes
            # from the s-pool ([128,1024] tiles, free once exps are done).
            op_tail_state = {}

            def op_tail_p1(rt):
                rs = slice(rt * 128, rt * 128 + 128)

                def th():
                    ps = ps_s.tile([128, D], f32, tag="s", name="opt")
                    op_tail_state[rt] = ps
                    for nj in range(D // 512):
                        ns = slice(nj * 512, nj * 512 + 512)
                        for khp in (0, 1):
                            nc.tensor.matmul(
                                ps[:, ns],
                                lhsT=ctx_sb[:, khp, rs],
                                rhs=wo_sb[:, khp, ns],
                                start=(khp == 0),
                                stop=False,
                            )

                return [th]

            def op_tail_p2(rt):
                rs = slice(rt * 128, rt * 128 + 128)

                def th():
                    ps = op_tail_state[rt]
                    for nj in range(D // 512):
                        ns = slice(nj * 512, nj * 512 + 512)
                        for khp in (2, 3):
                            nc.tensor.matmul(
                                ps[:, ns],
                                lhsT=ctx_sb[:, khp, rs],
                                rhs=wo_sb[:, khp, ns],
                                start=False,
                                stop=(khp == N_HP - 1),
                            )
                    ob = obp.tile([128, D], b16, tag="ob", name="obt")
                    if rt % 2 == 0:
                        nc.scalar.copy(out=ob[:], in_=ps[:])
                    else:
                        nc.vector.tensor_copy(out=ob[:], in_=ps[:])
                    nc.sync.dma_start(out_d[rs, :], ob[:])

                return [th]

            def q_key(hp, qc):
                return ("q", hp, qc)

            def k_key(hp, w):
                return ("k", hp, w)

            def v_key(mt, half):
                return ("v", mt, half)

            # prologue: only what gates the first exp
            force_emit = proj_qk_unit(wq_sb, wq1_sb, qt_sb, 0, 0, 512, S)
            for th in force_emit:
                th()
            kw0 = proj_qk_unit(wk_sb, wk1_sb, kt_sb, 0, 0, kwins[0][1], nkv)
            for th in kw0:
                th()

            # queue: rest of kt hp0, v half0, then hp1.. (force() is the net)
            for wi, (off, n) in enumerate(kwins[1:], start=1):
                add_unit(k_key(0, wi), proj_qk_unit(wk_sb, wk1_sb, kt_sb, 0, off, n, nkv))
            for mt in range(nkc):
                add_unit(v_key(mt, 0), v_unit(mt, 0))
            for hp in range(1, N_HP):
                add_unit(q_key(hp, 0), proj_qk_unit(wq_sb, wq1_sb, qt_sb, hp, 0, 512, S))
                for wi, (off, n) in enumerate(kwins):
                    add_unit(
                        k_key(hp, wi),
                        proj_qk_unit(wk_sb, wk1_sb, kt_sb, hp, off, n, nkv),
                    )
                if hp == 1:
                    for mt in range(nkc):
                        add_unit(v_key(mt, 1), v_unit(mt, 1))

            # ================= attention steps =================
            for t in range(4 * N_HP):
                qc, hp = divmod(t, N_HP)
                qs = slice(qc * QCH, qc * QCH + QCH)
                half = hp // 2

                # dependency safety: everything this step reads must be
                # emitted before its consumers
                force(q_key(hp, qc))
                for wi in range(len(kwins)):
                    force(k_key(hp, wi))
                for mt in range(nkc):
                    force(v_key(mt, half))

                # out-projection for row-tile (qc-1, hp): all head-pairs of
                # q-chunk qc-1 are complete by now.  Its chunks are placed
                # at explicit kc slots: the khp0,1 halves early, the khp2,3
                # halves (which wait on the previous step's norm chain,
                # running kc0-3 on VectorE) at kc5/kc7.
                slots = {}
                if qc > 0:
                    rt = (qc - 1) * N_HP + hp
                    op = op_unit(rt)
                    slots = {0: op["a0"], 5: op["b0"], 6: op["a1"], 7: op["b1"]}

                cc = ps_cc.tile([128, 2 * QCH], f32, tag="cc")
                c0 = cc[:, 0:QCH]
                c1 = cc[:, QCH : 2 * QCH]

                def ctx_mm(ekc, hp=hp, c0=c0, c1=c1):
                    e01_p, kc_p = ekc
                    nc.tensor.matmul(
                        c0,
                        lhsT=v_sb[:, kc_p, 2 * hp, :],
                        rhs=e01_p[:, 0:QCH],
                        start=(kc_p == 0),
                        stop=(kc_p == nkc - 1),
                    )
                    nc.tensor.matmul(
                        c1,
                        lhsT=v_sb[:, kc_p, 2 * hp + 1, :],
                        rhs=e01_p[:, QCH : 2 * QCH],
                        start=(kc_p == 0),
                        stop=(kc_p == nkc - 1),
                    )

                depth = nkc if t == 0 else 4
                pending = []
                for kc in range(nkc):
                    ks = slice(kc * KV_P, kc * KV_P + KV_P)
                    if kc in slots:
                        slots[kc]()
                    else:
                        fill(1)
                    s01 = ps_s.tile([128, 2 * QCH], f32, tag="s")
                    nc.tensor.matmul(
                        s01[:, 0:QCH],
                        lhsT=kt_sb[0:64, hp, ks],
                        rhs=qt_sb[0:64, hp, qs],
                    )
                    nc.tensor.matmul(
                        s01[:, QCH : 2 * QCH],
                        lhsT=kt_sb[64:128, hp, ks],
                        rhs=qt_sb[64:128, hp, qs],
                    )
                    e01 = ep.tile([128, 2 * QCH], b16, tag="e")
                    if kc >= 4 and kc % 2 == 0 and t < 4 * N_HP - 1:
                        # Schraudolph exp on VectorE (bf16 bits via int16)
                        nc.vector.tensor_scalar(
                            e01[:].bitcast(i16),
                            s01[:],
                            mbd_sb[:, kc : kc + 1],
                            0.0,
                            ADD,
                            MAX,
                        )
                    else:
                        # exact exp on ScalarE (scores pre-scaled by ALPHA)
                        nc.scalar.activation(
                            e01[:],
                            s01[:],
                            EXP,
                            bias=mba_sb[:, kc : kc + 1],
                            scale=1.0 / EXP_A,
                        )
                    pending.append((e01, kc))
                    if len(pending) > depth:
                        ctx_mm(pending.pop(0))
                for p in pending:
                    fill(2)
                    ctx_mm(p)

                # normalize: rows 64-127 of cc hold both heads' denominators
                # (replicated); relocate to base partition 0 (fast-reciprocal
                # breaks on shifted APs), one reciprocal, two multiplies.
                # Deferred into the next step's kc loop (see above) so the
                # serial chain doesn't head-of-line-block the DVE exps.
                def norm(hp=hp, qs=qs, cc=cc, c0=c0, c1=c1):
                    den01 = wp.tile([64, 2 * QCH], f32, tag="den", name="den")
                    nc.vector.tensor_copy(out=den01[:], in_=cc[64:128, :])
                    rc01 = wp.tile([64, 2 * QCH], f32, tag="rc", name="rc")
                    nc.vector.reciprocal_approx_fast(rc01[:], den01[:])
                    nc.vector.tensor_mul(
                        out=ctx_sb[0:64, hp, qs], in0=c0[0:64, :], in1=rc01[:, 0:QCH]
                    )
                    nc.vector.tensor_mul(
                        out=ctx_sb[64:128, hp, qs],
                        in0=c1[0:64, :],
                        in1=rc01[:, QCH : 2 * QCH],
                    )

                norm()

                # queue the q window needed a full qc ahead
                if qc < 3:
                    add_unit(
                        q_key(hp, qc + 1),
                        proj_qk_unit(
                            wq_sb, wq1_sb, qt_sb, hp, (qc + 1) * 512, 512, S
                        ),
                    )

            # drain: last q-chunk's out-projections (phase-split so the PE
            # stays warm across the final norm chain) + queue leftovers
            order = []
            for i in range(N_HP):
                rt = 3 * N_HP + i
                order.append(("opt1", rt))
                if i >= 1:
                    order.append(("opt2", rt - 1))
            order.append(("opt2", 3 * N_HP + N_HP - 1))
            for kind, rt in order:
                add_unit((kind, rt), op_tail_p1(rt) if kind == "opt1" else op_tail_p2(rt))
            while queue:
                fill(1)

    nc.finalize()
    return nc


def _pack(a, kc):
    """[kc*128, n] -> [128, kc*n] partition-major bf16 (SBUF layout)."""
    k128, n = a.shape
    return (
        np.ascontiguousarray(a.reshape(kc, 128, n).transpose(1, 0, 2))
        .reshape(128, kc * n)
        .astype(bf16)
    )


def _pack_w_hp(wT):
    """[D, DL] transposed weight -> [128, N_HP*KC8*128] hp-major."""
    a = wT.reshape(KC8, 128, N_HP, 128).transpose(1, 2, 0, 3)
    return np.ascontiguousarray(a).reshape(128, N_HP * KC8 * 128).astype(bf16)


def _host_prep(x, mask, wq, bq, wk, bk, wv, bv, wo):
    x = np.asarray(x, dtype=np.float32)
    mask = np.asarray(mask)
    idxs = [np.nonzero(mask[b])[0] for b in range(B)]
    nmax = max(1, max(len(i) for i in idxs))
    nkv = min(S, ((nmax + KV_P - 1) // KV_P) * KV_P)
    with_bias = bool(
        np.any(np.asarray(bq)) or np.any(np.asarray(bk)) or np.any(np.asarray(bv))
    )

    in_maps = []
    for c in range(DP * TP):
        b, g = c // TP, c % TP
        sl = slice(g * DL, g * DL + DL)

        idx = idxs[b]
        xg = np.zeros((nkv, D), dtype=np.float32)
        xg[: len(idx)] = x[b][idx]

        mba = np.full((nkv,), NEG, dtype=np.float32)
        mba[: len(idx)] = 0.0
        mbd = np.full((nkv,), NEG_DVE, dtype=np.float32)
        mbd[: len(idx)] = EXP_B

        im = {
            "xq": _pack(x[b].T, KC8),
            "xkv": _pack(xg.T, KC8),
            "wqt": _pack_w_hp(np.asarray(wq, dtype=np.float32)[sl, :].T * ALPHA),
            "wkt": _pack_w_hp(np.asarray(wk, dtype=np.float32)[sl, :].T),
            "wvt": _pack(np.asarray(wv)[sl, :].T, KC8),
            "wot": _pack(np.asarray(wo)[:, sl].T, DL // 128),
            "mbact": mba,
            "mbdve": mbd,
        }
        if with_bias:
            im["bq"] = (np.asarray(bq, dtype=np.float32)[None, sl] * ALPHA).astype(bf16)
            im["bk"] = np.asarray(bk)[None, sl].astype(bf16)
            im["bv"] = np.asarray(bv)[None, sl].astype(bf16)
        in_maps.append(im)
    return nkv, with_bias, in_maps


def kernel(x, mask, wq, bq, wk, bk, wv, bv, wo, bo):
    from concourse.bass_utils import run_bass_kernel_spmd

    nkv, with_bias, in_maps = _host_prep(x, mask, wq, bq, wk, bk, wv, bv, wo)
    nc = _build(nkv, with_bias)
    res = run_bass_kernel_spmd(nc, in_maps, core_ids=list(range(DP * TP)))

    out = np.empty((B, S, D), dtype=np.float32)
    bo = np.asarray(bo, dtype=np.float32)
    for b in range(B):
        out[b] = (
            res.results[b * TP]["out"].astype(np.float32)
            + res.results[b * TP + 1]["out"].astype(np.float32)
            + bo
        )
    return out
